# revision 23
# baseline (speedup 1.0000x reference)
"""Causal single-head attention (B=4, S=2048, D=1024) on 8 NeuronCores.

Sharding: core c owns the q rows {2i + (c%2)} of batch c//2 (1024 rows).
Interleaving q rows by parity gives every core an identical causal
block structure, so one SPMD program serves all 8 cores; only the data
(and the staircase mask) differs per core.

Key order is globally redefined as [parity-0 rows asc, parity-1 rows
asc] — attention is invariant to key permutation as long as K, V and
the mask agree. Under that order each core's q rows are its own parity
half, its causal extent per q-block j is the uniform tile set
[0, 4(j+1)) + [8, 8+4(j+1)) (128-key tiles), and exactly 8 tiles per
block cross the diagonal. Crossing tile with in-block offset c is
fully masked on its first 128*c q columns: scores/exp run only on the
remaining columns (left part memset to 0) and AV matmuls for
q-subtiles u < c are skipped.

K/V projections are deduplicated across the core pair of each batch:
core p computes K/V only for its parity rows, and the pair exchanges
halves with 2-core AllGathers (DRAM bounce), chunked so the collectives
and read-backs pipeline under the q projection and score matmuls.

Softmax denominators: rowsumT[1, sq] accumulates ones.T @ w per block
on the PE, is transposed to per-partition layout with K=1 matmuls, and
reciprocals scale the AV output.

All inputs arrive host-pre-tiled ([d, 128, n] contiguous 256KB tiles)
so DMA descriptors stay large; input DMA triggers are split across the
Sync and Scalar queues (each trigger costs ~0.6us of queue time).
"""

import sys
import types

import numpy as np
import ml_dtypes

import concourse.tile as tile
from concourse import bacc, mybir
from concourse.bass_utils import run_bass_kernel_spmd


def _ensure_ntff_hook():
    """bass_utils imports antenv.axon_hooks when tracing; some containers
    lack that module. Register a process-local equivalent so trace=True
    works (or degrades to untraced instead of crashing)."""
    try:
        import antenv.axon_hooks  # noqa: F401
        return
    except ImportError:
        pass
    hook = None
    try:
        from trn_agent_boot.trn_boot import _ntff_profile_via_ctypes
        hook = _ntff_profile_via_ctypes("/opt/axon/libaxon_pjrt.so")
    except Exception:
        hook = None
    mod = types.ModuleType("antenv.axon_hooks")
    mod.get_axon_ntff_profile_hook = lambda: hook
    mod.set_axon_ntff_profile_hook = lambda h: None
    sys.modules["antenv.axon_hooks"] = mod


_ensure_ntff_hook()

BF16 = mybir.dt.bfloat16
F32 = mybir.dt.float32
AF = mybir.ActivationFunctionType

B, S, D = 4, 2048, 1024
P = 128
NCORES = 8
SQ = 1024            # q rows per core (= own parity half)
ND = D // P          # 8 contraction tiles over d
NE = D // P          # 8 tiles over e (d_out)
NSK = S // P         # 16 key tiles
QB = 512             # q-block width (matmul free dim)
NQB = SQ // QB       # 2 q blocks
SCALE = 1.0 / np.sqrt(np.float32(D))
PAIRS = [[2 * b, 2 * b + 1] for b in range(B)]

TRACE = False
LAST_RESULT = None

_cache = {}


def _sk_list(j):
    # key tiles needed by q-block j: prefix of each parity half
    return list(range(0, 4 * (j + 1))) + list(range(8, 8 + 4 * (j + 1)))


def _cross_list(j):
    # diagonal-crossing key tiles of q-block j (order matches maskd[j])
    return list(range(4 * j, 4 * (j + 1))) + list(range(8 + 4 * j, 8 + 4 * (j + 1)))


def _coff(j, t):
    # in-block crossing offset: first 128*c q columns of tile t are fully
    # masked within q-block j (c = 0 for non-crossing computed tiles)
    return max(0, (t % 8) - 4 * j)


def _build():
    nc = bacc.Bacc("TRN2", target_bir_lowering=False, debug=False,
                   num_devices=NCORES)
    xot = nc.dram_tensor("xot", [ND, P, SQ], BF16, kind="ExternalInput")
    wqt = nc.dram_tensor("wqt", [ND, P, D], BF16, kind="ExternalInput")
    wkt = nc.dram_tensor("wkt", [ND, P, D], BF16, kind="ExternalInput")
    wvt = nc.dram_tensor("wvt", [ND, P, D], BF16, kind="ExternalInput")
    maskd = nc.dram_tensor("maskd", [P, NQB, 8, QB], BF16, kind="ExternalInput")
    ones = nc.dram_tensor("ones", [P, 8], BF16, kind="ExternalInput")
    out = nc.dram_tensor("out", [SQ, D], F32, kind="ExternalOutput")

    from contextlib import ExitStack
    with tile.TileContext(nc) as tc:
        with ExitStack() as ctx:
            xo_pool = ctx.enter_context(tc.tile_pool(name="xo", bufs=ND))
            wk_pool = ctx.enter_context(tc.tile_pool(name="wk", bufs=ND))
            wv_pool = ctx.enter_context(tc.tile_pool(name="wv", bufs=ND))
            wq_pool = ctx.enter_context(tc.tile_pool(name="wq", bufs=ND))
            st_pool = ctx.enter_context(tc.tile_pool(name="st", bufs=6))
            kT_pool = ctx.enter_context(tc.tile_pool(name="kT", bufs=1))
            v_pool = ctx.enter_context(tc.tile_pool(name="v", bufs=1))
            qT_pool = ctx.enter_context(tc.tile_pool(name="qT", bufs=NE))
            m_pool = ctx.enter_context(tc.tile_pool(name="mk", bufs=1))
            we_pool = ctx.enter_context(tc.tile_pool(name="we", bufs=16))
            on_pool = ctx.enter_context(tc.tile_pool(name="on", bufs=2))
            sm_pool = ctx.enter_context(tc.tile_pool(name="sm", bufs=2))
            rc_pool = ctx.enter_context(tc.tile_pool(name="rc", bufs=4))
            o_pool = ctx.enter_context(tc.tile_pool(name="o", bufs=2))
            dr_pool = ctx.enter_context(
                tc.tile_pool(name="dr", bufs=8, space="DRAM"))
            ps_pool = ctx.enter_context(
                tc.tile_pool(name="ps", bufs=2, space="PSUM"))
            av_pool = ctx.enter_context(
                tc.tile_pool(name="av", bufs=2, space="PSUM"))
            rs_pool = ctx.enter_context(
                tc.tile_pool(name="rs", bufs=2, space="PSUM"))
            # ---- input DMAs ----
            # critical path (stage A): xo + wk interleaved on the sync queue
            xo, wk_t = [], []
            for d in range(ND):
                t = xo_pool.tile([P, SQ], BF16, tag="xo")
                nc.sync.dma_start(t[:], xot[d])
                xo.append(t)
                t = wk_pool.tile([P, D], BF16, tag="wk")
                nc.sync.dma_start(t[:], wkt[d])
                wk_t.append(t)
            # later-stage inputs are triggered from the scalar queue and
            # gated behind stage A's first psum group (see below) so the
            # ramp-critical xo+wk transfers get the full DMA bandwidth
            wv_t, wq_t = [], []
            deferred = []
            for d in range(ND):
                t = wv_pool.tile([P, D], BF16, tag="wv")
                deferred.append(nc.scalar.dma_start(t[:], wvt[d]))
                wv_t.append(t)
                t = wq_pool.tile([P, D], BF16, tag="wq")
                deferred.append(nc.scalar.dma_start(t[:], wqt[d]))
                wq_t.append(t)
            mask_big = m_pool.tile([P, NQB, 8, QB], BF16, tag="mk")
            deferred.append(nc.scalar.dma_start(mask_big[:], maskd[:]))
            ones_t = on_pool.tile([P, 8], BF16, tag="on")
            deferred.append(nc.scalar.dma_start(ones_t[:], ones[:]))

            kT_big = kT_pool.tile([P, NE, S], BF16, tag="kT")
            v_big = v_pool.tile([P, NSK, D], BF16, tag="v")

            warm = st_pool.tile([P, P], BF16, tag="warm")
            nc.vector.memset(warm[:], 0.0)
            wps = ps_pool.tile([P, P], F32, tag="ps")
            for i in range(72):
                nc.tensor.matmul(wps[:], warm[:], warm[:],
                                 start=(i == 0), stop=(i == 71))

            # ---- stage A: kT own half [e, s0], exchanged in 2 E-chunks ----
            for h in range(2):
                ex_in = dr_pool.tile([4, P, SQ], BF16, tag=f"exik{h}")
                ex_out = dr_pool.tile([2, 4, P, SQ], BF16, tag=f"exok{h}")
                # waves of 4 concurrent psum groups (2 ps + 2 borrowed
                # av-pool banks), d-outermost: during the input-DMA ramp the
                # PE advances every open group as each d tile lands instead
                # of stalling on one group's full reduction
                for wave in range(2):
                    ksts = [st_pool.tile([P, SQ], BF16, tag="st",
                                         name=f"kst{h}{wave}{i}")
                            for i in range(2)]
                    grp = [(Eo, Sc) for Eo in (2 * wave, 2 * wave + 1)
                           for Sc in range(SQ // QB)]
                    pss = [ps_pool.tile([P, QB], F32, tag="ps",
                                        name=f"aps{h}{wave}{g}")
                           if g < 2 else
                           av_pool.tile([P, QB], F32, tag="av",
                                        name=f"aav{h}{wave}{g}")
                           for g in range(4)]
                    for d in range(ND):
                        for g, (Eo, Sc) in enumerate(grp):
                            E = 4 * h + Eo
                            mm = nc.tensor.matmul(
                                pss[g][:],
                                wk_t[d][:, E * P:(E + 1) * P],
                                xo[d][:, Sc * QB:(Sc + 1) * QB],
                                start=(d == 0), stop=(d == ND - 1),
                            )
                        if h == 0 and wave == 0 and d == ND - 1:
                            from concourse.bass import _add_dep_helper
                            for dd in deferred:
                                _add_dep_helper(
                                    dd.ins, mm.ins, sync=True,
                                    reason="defer bulk loads past ramp")
                    for g, (Eo, Sc) in enumerate(grp):
                        nc.vector.tensor_copy(
                            ksts[Eo - 2 * wave][:, Sc * QB:(Sc + 1) * QB],
                            pss[g][:])
                    for i in range(2):
                        nc.sync.dma_start(ex_in[2 * wave + i], ksts[i][:])
                nc.gpsimd.collective_compute(
                    "AllGather", mybir.AluOpType.bypass, replica_groups=PAIRS,
                    ins=[ex_in.opt()], outs=[ex_out.opt()],
                )
                for r in range(2):
                    for i in range(2):
                        nc.sync.dma_start(
                            kT_big[:, 4 * h + 2 * i:4 * h + 2 * (i + 1),
                                   r * SQ:(r + 1) * SQ],
                            ex_out[r, 2 * i:2 * (i + 1)].rearrange(
                                "n p m -> p n m"))

            # ---- stage B: v own half [s0, e], exchanged in 2 s-chunks ----
            for h in range(2):
                ex_in = dr_pool.tile([4, P, D], BF16, tag=f"exiv{h}")
                ex_out = dr_pool.tile([2, 4, P, D], BF16, tag=f"exov{h}")
                for so in range(4):
                    sT = 4 * h + so
                    vst = st_pool.tile([P, D], BF16, tag="st")
                    for ec in range(D // QB):
                        ps = ps_pool.tile([P, QB], F32, tag="ps")
                        for d in range(ND):
                            nc.tensor.matmul(
                                ps[:],
                                xo[d][:, sT * P:(sT + 1) * P],
                                wv_t[d][:, ec * QB:(ec + 1) * QB],
                                start=(d == 0), stop=(d == ND - 1),
                            )
                        nc.vector.tensor_copy(
                            vst[:, ec * QB:(ec + 1) * QB], ps[:])
                    nc.sync.dma_start(ex_in[so], vst[:])
                nc.gpsimd.collective_compute(
                    "AllGather", mybir.AluOpType.bypass, replica_groups=PAIRS,
                    ins=[ex_in.opt()], outs=[ex_out.opt()],
                )
                for r in range(2):
                    for i in range(2):
                        nc.sync.dma_start(
                            v_big[:, 8 * r + 4 * h + 2 * i:
                                  8 * r + 4 * h + 2 * (i + 1), :],
                            ex_out[r, 2 * i:2 * (i + 1)].rearrange(
                                "n p m -> p n m"))

            # ---- stage C: qT[e, i] from own rows ----
            qT_t = []
            for E in range(NE):
                t = qT_pool.tile([P, SQ], BF16, tag="qT")
                qT_t.append(t)
            for E in range(NE):
                for qc in range(SQ // QB):
                    ps = ps_pool.tile([P, QB], F32, tag="ps")
                    for d in range(ND):
                        nc.tensor.matmul(
                            ps[:],
                            wq_t[d][:, E * P:(E + 1) * P],
                            xo[d][:, qc * QB:(qc + 1) * QB],
                            start=(d == 0), stop=(d == ND - 1),
                        )
                    nc.vector.tensor_copy(qT_t[E][:, qc * QB:(qc + 1) * QB], ps[:])

            # ---- stage D: attention per q block ----
            for j in range(NQB):
                sk_list = _sk_list(j)
                cross = _cross_list(j)
                wtiles = {}
                for t in sk_list:
                    c = _coff(j, t)
                    w0 = c * P          # first live q column of this tile
                    ps = ps_pool.tile([P, QB], F32, tag="ps")
                    for E in range(NE):
                        nc.tensor.matmul(
                            ps[:, 0:QB - w0],
                            kT_big[:, E, t * P:(t + 1) * P],
                            qT_t[E][:, j * QB + w0:(j + 1) * QB],
                            start=(E == 0), stop=(E == NE - 1),
                        )
                    wt = we_pool.tile([P, QB], BF16, tag="we")
                    nc.scalar.activation(wt[:, w0:QB], ps[:, 0:QB - w0],
                                         AF.Exp, scale=float(SCALE))
                    if t in cross:
                        tt = cross.index(t)
                        nc.vector.tensor_mul(wt[:, w0:QB], wt[:, w0:QB],
                                             mask_big[:, j, tt, w0:QB])
                    wtiles[t] = wt

                for u in range(QB // P):
                    ts_u = sorted(
                        (t for t in sk_list if _coff(j, t) <= u),
                        key=lambda t: ((t % 8) >= 4, t))
                    av = av_pool.tile([P, D], F32, tag="av")
                    rs = rs_pool.tile([P, 1], F32, tag="rs")
                    n = len(ts_u)
                    for idx, t in enumerate(ts_u):
                        lhsT = wtiles[t][:, u * P:(u + 1) * P]
                        st, sp = idx == 0, idx == n - 1
                        nc.tensor.matmul(av[:, 0:QB], lhsT, v_big[:, t, 0:QB],
                                         start=st, stop=sp)
                        nc.tensor.matmul(av[:, QB:D], lhsT, v_big[:, t, QB:D],
                                         start=st, stop=sp)
                        nc.tensor.matmul(rs[:], lhsT, ones_t[:, 0:1],
                                         start=st, stop=sp)
                    rcp = rc_pool.tile([P, 1], F32, tag="rcp")
                    nc.vector.reciprocal(rcp[:], rs[:])
                    ot = o_pool.tile([P, D], F32, tag="o")
                    r0 = (j * (QB // P) + u) * P
                    for eh in range(2):
                        nc.vector.tensor_scalar_mul(
                            ot[:, eh * QB:(eh + 1) * QB],
                            av[:, eh * QB:(eh + 1) * QB], rcp[:])
                        nc.sync.dma_start(out[r0:r0 + P, eh * QB:(eh + 1) * QB],
                                          ot[:, eh * QB:(eh + 1) * QB])

    nc.compile()
    return nc


def _prep_inputs(x, Wq, Wk, Wv):
    bf = ml_dtypes.bfloat16

    def tiled(a):     # [D, n] -> [ND, P, n] contiguous
        return np.ascontiguousarray(
            a.reshape(ND, P, a.shape[1]).astype(bf))

    # weights are used as lhsT in natural [d, e] layout
    wq_b = tiled(Wq)
    wk_b = tiled(Wk)
    wv_b = tiled(Wv)
    ones = np.ones((P, 8), bf)
    ks = np.arange(S)
    ii = np.arange(SQ)
    # global index of permuted key position (parity-0 rows, then parity-1)
    gk = np.where(ks < SQ, 2 * ks, 2 * (ks - SQ) + 1)
    in_maps = []
    for c in range(NCORES):
        b, p = c // 2, c % 2
        xoT = x[b, p::2].T                          # [D, SQ]
        gq = 2 * ii + p
        maskd = np.zeros((NQB, 8, P, QB), np.float32)
        for j in range(NQB):
            for tt, t in enumerate(_cross_list(j)):
                gk_t = gk[t * P:(t + 1) * P]
                gq_j = gq[QB * j:QB * (j + 1)]
                maskd[j, tt] = (gk_t[:, None] <= gq_j[None, :]).astype(np.float32)
        # device layout [P, NQB, 8, QB] (partition-major, contiguous rows)
        mask_dev = np.ascontiguousarray(
            maskd.transpose(2, 0, 1, 3).astype(bf))
        in_maps.append({
            "xot": tiled(xoT), "wqt": wq_b, "wkt": wk_b, "wvt": wv_b,
            "maskd": mask_dev, "ones": ones,
        })
    return in_maps


def kernel(x, Wq, Wk, Wv):
    global LAST_RESULT
    x = np.asarray(x, np.float32)
    Wq = np.asarray(Wq, np.float32)
    Wk = np.asarray(Wk, np.float32)
    Wv = np.asarray(Wv, np.float32)

    if "nc" not in _cache:
        _cache["nc"] = _build()
    nc = _cache["nc"]

    in_maps = _prep_inputs(x, Wq, Wk, Wv)
    res = run_bass_kernel_spmd(nc, in_maps, list(range(NCORES)), trace=TRACE)
    LAST_RESULT = res

    out = np.empty((B, S, D), np.float32)
    for c in range(NCORES):
        b, p = c // 2, c % 2
        out[b, p::2, :] = res.results[c]["out"]
    return out
